# revision 32
# baseline (speedup 1.0000x reference)
"""GQA attention kernel for Trainium2, 8-core tensor-parallel.

Sharding: 8 cores = 2 batches x 4 KV-groups. Each core handles one
(batch, kv_group): projections for its 4 Q-heads + 1 KV-head, RoPE,
causal attention, and its row-shard of Wo -> partial [T, D] output.
Host sums the 4 partials per batch (the Wo all-reduce) at unshard.

v3: two-phase schedule. Phase 1 runs K/V projections (+RoPE, V
transpose) for ALL four q-tiles back-to-back — dense PE work that
matches the input-stream bandwidth, keeping the HAM clock-gate warm
through the DMA-bound startup. Phase 2 runs per-tile Q-projections,
attention, and the output projection with x fully resident in SBUF (no
input DMA at all); the next tile's first Q-projection covers the last
head's sigma chain at each tile boundary. Within a head, the S-matmul
batch runs one step ahead of the O-matmuls and the next head's Q-proj
fills the scalar-engine exp latency; the sigma ones-matmul is deferred
one head. All matmuls bf16; sigma accumulates on the vector engine in
bf16 2x mode in a two-stripe [128,1024] accumulator; diagonal S/O
matmuls + exp are narrowed to the valid query range; exp batches
s-chunk pairs into [128,1024] ACTIVATEs. Output partials are written
bf16 across two DMA queues.
"""
from contextlib import ExitStack

import numpy as np

import concourse.mybir as mybir
import concourse.tile as tile
from concourse import bacc
from concourse.bass_utils import run_bass_kernel_spmd

B, T, D = 2, 2048, 2048
H, KV, HD = 16, 4, 128
R = H // KV                  # 4 query heads per kv head (per core)
GC = R * HD                  # 512 query-proj cols per core
THETA = 10000.0
TQ = 512                     # q-tile size
NJ = T // TQ                 # 4 q-tiles
ND = D // 128                # 16 contraction chunks
SCALE = float(HD) ** -0.5

F32 = mybir.dt.float32
BF16 = mybir.dt.bfloat16
AF = mybir.ActivationFunctionType

_CACHED_NC = None


def _build_nc():
    nc = bacc.Bacc("TRN2", target_bir_lowering=False, debug=False, num_devices=8)

    # xT: chunk d at cols [d*T, (d+1)*T), natural token order within chunk
    xT = nc.dram_tensor("xT", [128, ND * T], BF16, kind="ExternalInput").ap()
    wq = nc.dram_tensor("wq", [128, ND * GC], BF16, kind="ExternalInput").ap()
    wk = nc.dram_tensor("wk", [128, ND * HD], BF16, kind="ExternalInput").ap()
    wv = nc.dram_tensor("wv", [128, ND * HD], BF16, kind="ExternalInput").ap()
    wo = nc.dram_tensor("wo", [128, R * D], BF16, kind="ExternalInput").ap()
    cosT = nc.dram_tensor("cosT", [HD, T], BF16, kind="ExternalInput").ap()
    sinT = nc.dram_tensor("sinT", [HD, T], BF16, kind="ExternalInput").ap()
    tri = nc.dram_tensor("tri", [128, 128], BF16, kind="ExternalInput").ap()
    out = nc.dram_tensor("out", [T, D], BF16, kind="ExternalOutput").ap()

    with tile.TileContext(nc) as tc, ExitStack() as ctx:
        res = ctx.enter_context(tc.tile_pool(name="res", bufs=1))
        sb = ctx.enter_context(tc.tile_pool(name="sb", bufs=2))
        pp = ctx.enter_context(tc.tile_pool(name="pp", bufs=2, space="PSUM"))

        # ---- warm-up source (no DMA dependency) + PE HAM warm-up ----
        warm = res.tile([128, 512], BF16)
        nc.vector.memset(warm[:], 0.0)
        ones_c = res.tile([128, 128], BF16)
        nc.vector.memset(ones_c[:], 1.0)
        ident = res.tile([128, 128], BF16)
        from concourse.masks import make_identity
        identf = res.tile([128, 128], F32)
        make_identity(nc, identf[:])
        nc.vector.tensor_copy(ident[:], identf[:])
        # preload the exp activation table while the PE warms up
        dume = res.tile([1, 8], F32)
        nc.vector.memset(dume[:], 0.0)
        nc.scalar.activation(dume[:], dume[:], AF.Exp, scale=1.0)
        for w in range(16):
            wm_ps = pp.tile([128, 512], F32, tag="ps", bufs=2, name=f"warm{w}")
            nc.tensor.matmul(wm_ps[:], ones_c[:], warm[:], start=True, stop=True)

        # ---- resident weights / tables ----
        wk_sb = res.tile([128, ND * HD], BF16)
        nc.scalar.dma_start(wk_sb[:], wk[:])
        # wq laid out per head: head h chunk d at cols [h*ND*HD + d*HD, ...)
        wq_sb = res.tile([128, R * ND * HD], BF16)
        for hh in range(R):
            nc.scalar.dma_start(wq_sb[:, hh * ND * HD:(hh + 1) * ND * HD],
                                wq[:, hh * ND * HD:(hh + 1) * ND * HD])
        wo_sb = res.tile([128, R * D], BF16)     # head h rows at cols [h*D,(h+1)*D)
        nc.scalar.dma_start(wo_sb[:], wo[:])
        wv_sb = res.tile([128, ND * HD], BF16)
        nc.gpsimd.dma_start(wv_sb[:], wv[:])
        tri_sb = res.tile([128, 128], BF16)
        nc.gpsimd.dma_start(tri_sb[:], tri[:])

        kT_sb = res.tile([128, T], BF16)         # K^T resident
        v_sb = res.tile([128, T], BF16)          # V natural, chunk c at cols c*128
        cos_sb = res.tile([128, T], BF16)        # rope tables resident
        sin_sb = res.tile([128, T], BF16)

        # ---- stage ALL x tiles + rope tables up front ----
        xts_all = []
        for j in range(NJ):
            q0 = j * TQ
            xts = []
            for d in range(ND):
                xt = sb.tile([128, TQ], BF16, tag="xt", bufs=64, name=f"xt{j}_{d}")
                eng = nc.sync if d % 2 == 0 else nc.gpsimd
                eng.dma_start(xt[:], xT[:, d * T + q0:d * T + q0 + TQ])
                xts.append(xt)
            xts_all.append(xts)
            if j == 0:
                nc.sync.dma_start(cos_sb[:], cosT[:])
                nc.sync.dma_start(sin_sb[:], sinT[:])

        def rope(dst, ps, j):
            # dst = ps * cos + rotate_half(ps) * sin  (partition dim = head dim)
            q0 = j * TQ
            rot = sb.tile([128, TQ], BF16, tag="rot", bufs=2)
            nc.scalar.mul(rot[0:64, :], ps[64:128, :], -1.0)
            nc.scalar.copy(rot[64:128, :], ps[0:64, :])
            tmp = sb.tile([128, TQ], F32, tag="ropetmp", bufs=2)
            nc.vector.tensor_mul(tmp[:], rot[:], sin_sb[:, q0:q0 + TQ])
            m1 = sb.tile([128, TQ], F32, tag="ropem1", bufs=2)
            nc.vector.tensor_mul(m1[:], ps[:], cos_sb[:, q0:q0 + TQ])
            nc.vector.tensor_add(dst, m1[:], tmp[:])

        def q_proj(h, j):
            q_ps = pp.tile([128, TQ], F32, tag="pa", bufs=2)
            for d in range(ND):
                nc.tensor.matmul(
                    q_ps[:], wq_sb[:, h * ND * HD + d * HD:h * ND * HD + (d + 1) * HD],
                    xts_all[j][d][:], start=(d == 0), stop=(d == ND - 1))
            qh = sb.tile([128, TQ], BF16, tag="qsb", bufs=3)
            rope(qh[:], q_ps, j)
            return qh

        def kv_proj_mm(j):
            """K^T (+rope) into kT_sb, V^T staged for the transposes."""
            q0 = j * TQ
            k_ps = pp.tile([128, TQ], F32, tag="pa", bufs=2)
            vt_ps = pp.tile([128, TQ], F32, tag="pa", bufs=2)
            for d in range(ND):
                nc.tensor.matmul(k_ps[:], wk_sb[:, d * HD:(d + 1) * HD],
                                 xts_all[j][d][:],
                                 start=(d == 0), stop=(d == ND - 1))
            for d in range(ND):
                nc.tensor.matmul(vt_ps[:], wv_sb[:, d * HD:(d + 1) * HD],
                                 xts_all[j][d][:],
                                 start=(d == 0), stop=(d == ND - 1))
            vt_sbt = sb.tile([128, TQ], BF16, tag="vtsb", bufs=2)
            nc.vector.tensor_copy(vt_sbt[:], vt_ps[:])
            rope(kT_sb[:, q0:q0 + TQ], k_ps, j)
            return vt_sbt

        def v_transpose(j, vt_sbt):
            for c4 in range(4):
                ptt = pp.tile([128, 128], BF16, tag="pa", bufs=2)
                nc.tensor.transpose(ptt[:], vt_sbt[:, c4 * 128:(c4 + 1) * 128], ident[:])
                nc.vector.tensor_copy(v_sb[:, (4 * j + c4) * 128:(4 * j + c4 + 1) * 128], ptt[:])

        # ==== phase 1: K/V projections for all tiles, back-to-back ====
        vt_prev = None
        for j in range(NJ):
            vt_cur = kv_proj_mm(j)
            if vt_prev is not None:
                v_transpose(j - 1, vt_prev)
            vt_prev = vt_cur
        v_transpose(NJ - 1, vt_prev)

        # ==== phase 2: per tile: Q-proj + attention + out-proj ====
        qh_cur = q_proj(0, 0)
        for j in range(NJ):
            q0 = j * TQ
            ncf = 4 * j              # full (below-diagonal) s-chunks

            o_tiles = []

            def finish_head(st):
                """sigma ones-MM + reciprocal + normalize for a finished head."""
                acc2, o_ps = st
                sg_ps = pp.tile([128, TQ], F32, tag="pa", bufs=2)
                nc.tensor.matmul(sg_ps[:], ones_c[:], acc2[:, 0:TQ],
                                 start=True, stop=True)
                rcb = sb.tile([128, TQ], F32, tag="rcb", bufs=2)
                nc.vector.reciprocal_approx_fast(rcb[:], sg_ps[:])
                oh = sb.tile([128, TQ], BF16, tag="osb", bufs=6)
                nc.vector.tensor_mul(oh[:], o_ps[:], rcb[:])
                o_tiles.append(oh)

            def make_batches(qh, o_ps, ctx_h):
                """Batches: paired full chunks then diagonal chunks; each entry
                (emit_S, emit_O) shares a per-batch cell."""
                batches = []
                for bi in range(ncf // 2):
                    cell = {}

                    def eS(bi=bi, cell=cell):
                        c0 = 2 * bi
                        s_grp = pp.tile([128, 2 * TQ], F32, tag="ps", bufs=2)
                        for cc in range(2):
                            nc.tensor.matmul(s_grp[:, cc * TQ:(cc + 1) * TQ],
                                             kT_sb[:, (c0 + cc) * 128:(c0 + cc + 1) * 128],
                                             qh[:], start=True, stop=True)
                        p_grp = sb.tile([128, 2 * TQ], BF16, tag="psb", bufs=4)
                        nc.scalar.activation(p_grp[:], s_grp[:], AF.Exp, scale=SCALE)
                        cell['p'] = p_grp

                    def eO(bi=bi, cell=cell):
                        c0 = 2 * bi
                        p_grp = cell.pop('p')
                        for cc in range(2):
                            nc.tensor.matmul(o_ps[:],
                                             v_sb[:, (c0 + cc) * 128:(c0 + cc + 1) * 128],
                                             p_grp[:, cc * TQ:(cc + 1) * TQ],
                                             start=(c0 + cc == 0), stop=False)
                        if 'acc' not in ctx_h:
                            ctx_h['acc'] = sb.tile([128, 2 * TQ], BF16, tag="acc2",
                                                   bufs=2, name="acc2")
                            nc.vector.tensor_copy(ctx_h['acc'][:], p_grp[:])
                        else:
                            nc.vector.tensor_add(ctx_h['acc'][:], ctx_h['acc'][:], p_grp[:])
                    batches.append((eS, eO))
                for m in range(4):
                    cell = {}

                    def eS(m=m, cell=cell):
                        w = TQ - m * 128
                        c = 4 * j + m
                        s_d = pp.tile([128, 2 * TQ], F32, tag="ps", bufs=2)
                        nc.tensor.matmul(s_d[:, 0:w],
                                         kT_sb[:, c * 128:(c + 1) * 128],
                                         qh[:, m * 128:TQ], start=True, stop=True)
                        p_d = sb.tile([128, TQ], BF16, tag="psb", bufs=4)
                        nc.scalar.activation(p_d[:, 0:w], s_d[:, 0:w], AF.Exp,
                                             scale=SCALE)
                        nc.vector.tensor_mul(p_d[:, 0:128], p_d[:, 0:128], tri_sb[:])
                        cell['p'] = p_d

                    def eO(m=m, cell=cell):
                        w = TQ - m * 128
                        c = 4 * j + m
                        p_d = cell.pop('p')
                        nc.tensor.matmul(o_ps[:, m * 128:TQ],
                                         v_sb[:, c * 128:(c + 1) * 128],
                                         p_d[:, 0:w],
                                         start=(c == 0), stop=(m == 3))
                        if 'acc' not in ctx_h:
                            # j == 0, m == 0: initialize the low stripe
                            ctx_h['acc'] = sb.tile([128, 2 * TQ], BF16, tag="acc2",
                                                   bufs=2, name="acc2")
                            nc.vector.tensor_copy(ctx_h['acc'][:, 0:TQ], p_d[:])
                        else:
                            nc.vector.tensor_add(ctx_h['acc'][:, m * 128:TQ],
                                                 ctx_h['acc'][:, m * 128:TQ],
                                                 p_d[:, 0:w])
                    batches.append((eS, eO))
                return batches

            pending = None
            for h in range(R):
                o_ps = pp.tile([128, TQ], F32, tag="po", bufs=2)
                ctx_h = {}
                batches = make_batches(qh_cur, o_ps, ctx_h)
                # first S batch, then fill the exp latency with the next
                # head's Q-projection
                batches[0][0]()
                if h + 1 < R:
                    qh_cur = q_proj(h + 1, j)
                if pending is not None:
                    finish_head(pending)
                for i in range(1, len(batches)):
                    batches[i][0]()
                    batches[i - 1][1]()
                batches[-1][1]()
                acc2 = ctx_h['acc']
                if j > 0:
                    nc.vector.tensor_add(acc2[:, 0:TQ], acc2[:, 0:TQ],
                                         acc2[:, TQ:2 * TQ])
                pending = (acc2, o_ps)

            # last head's sigma chain is covered by the next tile's Q-proj
            finish_head(pending)
            if j + 1 < NJ:
                qh_cur = q_proj(0, j + 1)

            # ---- out-proj for tile j ----
            for qs in range(4):
                for n in range(NJ):
                    pc = pp.tile([128, 512], F32, tag="po", bufs=2)
                    for h in range(R):
                        nc.tensor.matmul(
                            pc[:], o_tiles[h][:, qs * 128:(qs + 1) * 128],
                            wo_sb[:, h * D + n * 512:h * D + (n + 1) * 512],
                            start=(h == 0), stop=(h == R - 1))
                    ob = sb.tile([128, 512], BF16, tag="ob", bufs=4)
                    k = qs * NJ + n
                    if k % 2 == 0:
                        nc.scalar.copy(ob[:], pc[:])
                    else:
                        nc.vector.tensor_copy(ob[:], pc[:])
                    eng = (nc.gpsimd, nc.sync)[k % 2]
                    eng.dma_start(
                        out[q0 + qs * 128:q0 + (qs + 1) * 128, n * 512:(n + 1) * 512],
                        ob[:])

    nc.compile()
    return nc


def _get_nc():
    global _CACHED_NC
    if _CACHED_NC is None:
        _CACHED_NC = _build_nc()
    return _CACHED_NC


def _rope_tables_T():
    inv_freq = (1.0 / (THETA ** (np.arange(0, HD, 2, dtype=np.float32) / HD))).astype(np.float32)
    pos = np.arange(T, dtype=np.float32)
    freqs = np.outer(pos, inv_freq).astype(np.float32)      # [T, HD/2]
    emb = np.concatenate([freqs, freqs], axis=-1)           # [T, HD]
    return (np.cos(emb).T.copy(), np.sin(emb).T.copy())     # [HD, T] f32


def kernel(x, Wq, Wk, Wv, Wo, _trace=False):
    import ml_dtypes
    BFNP = ml_dtypes.bfloat16
    x = np.asarray(x, dtype=np.float32)

    cosT, sinT = _rope_tables_T()
    # tri[i, jj] = 1 if jj >= i (keep) else 0, for the diagonal 128-block
    i_ = np.arange(128)[:, None]
    jj_ = np.arange(128)[None, :]
    tri = (jj_ >= i_).astype(BFNP)

    def chunkT(w):  # [ND*128, C] -> [128, ND*C] with chunk d at cols [d*C,(d+1)*C)
        nd = w.shape[0] // 128
        return np.ascontiguousarray(
            w.reshape(nd, 128, -1).transpose(1, 0, 2).reshape(128, -1)).astype(BFNP)

    in_maps = []
    for core in range(8):
        b, g = core // KV, core % KV
        xb = x[b].T.reshape(ND, 128, T).transpose(1, 0, 2).reshape(128, ND * T)
        # wq per-head: [128, R*ND*128], head h chunk d at cols h*ND*128 + d*128
        wq_g = np.asarray(Wq)[:, g * GC:(g + 1) * GC]       # [D, R*HD]
        wq_heads = np.concatenate(
            [chunkT(wq_g[:, hh * HD:(hh + 1) * HD]) for hh in range(R)], axis=1)
        in_maps.append({
            "xT": np.ascontiguousarray(xb).astype(BFNP),
            "wq": np.ascontiguousarray(wq_heads),
            "wk": chunkT(np.asarray(Wk)[:, g * HD:(g + 1) * HD]),
            "wv": chunkT(np.asarray(Wv)[:, g * HD:(g + 1) * HD]),
            "wo": chunkT(np.asarray(Wo)[g * GC:(g + 1) * GC, :]),
            "cosT": cosT.astype(BFNP), "sinT": sinT.astype(BFNP), "tri": tri,
        })

    nc = _get_nc()
    res = run_bass_kernel_spmd(nc, in_maps, core_ids=list(range(8)), trace=_trace)

    outp = np.zeros((B, T, D), dtype=np.float32)
    for core in range(8):
        b = core // KV
        outp[b] += res.results[core]["out"].astype(np.float32)
    if _trace:
        kernel._last_exec_time_ns = res.exec_time_ns
        kernel._last_trace = res.instructions_and_trace
    return outp
